# revision 7
# baseline (speedup 1.0000x reference)
"""Trainium2 Bass kernel v2: correlation (cost volume) layer.

kernel(in1, in2): full inputs [8, 256, 96, 192] f32 -> output [8, 25, 96, 192] f32.
Sharding: data-parallel over batch, one batch per NeuronCore (8 cores, SPMD).

out[d=(a,b), h, w] = mean_c in1[c,h,w] * in2pad[c, h+2a-4, w+2b-4],  a,b in 0..4

v2 design vs baseline:
- fp16 matmul inputs (1 cyc/row vs fp32's 4; FWL weight-load overlap).
- Tall tiles: TH=16 rows x TW=8 cols on the parity grid -> 8 j-blocks
  (16-partition shear DMAs) instead of 16, 3 strips of 32 rows instead
  of 6 of 16.
- Band extraction via DRAM shear scratch in fp16 with tile-packed
  columns: per (partition, parity) the 12 tiles' 100-col windows abut
  -> one contiguous 2400B descriptor run per partition (full DMA bw),
  and the whole readback is one DMA per parity-pair.
- Shear fused over (tile, parity-pair): 8 shear DMAs + 1 readback per
  pair; ~70 DMAs total vs ~440 (DMA issue costs ~0.6us sequencer each).
- in1/in2 loaded once in chunks (no halo re-read); pk2 packs read
  across chunk boundaries.
- Pack work split across scalar/vector/gpsimd engines, weighted by
  measured per-engine strided-copy rates (vector ~1.4 ns/el, scalar
  ~1.7, gpsimd ~3.5).
- Emission software-pipelined: strip s's post-processing (compact,
  transposes, assemble) is emitted inside strip s+1 so the PE never
  queues dep-blocked transposes ahead of ready matmuls.
- Assemble APs iterate j (stride-2) innermost instead of i (stride-2W):
  the strided scatter onto 25 partitions drops from ~4.6 ns/el to
  ~2.9 ns/el on DVE (profiled), trimming the post-processing tail.
"""
import sys
if '/opt/trn_rl_repo' not in sys.path:
    sys.path.insert(0, '/opt/trn_rl_repo')
import numpy as np

import concourse.bass as bass
import concourse.mybir as mybir

f32 = mybir.dt.float32
f16 = mybir.dt.float16

C_FULL, H_FULL, W_FULL = 256, 96, 192
B_FULL = 8

TH, TW = 16, 8            # parity-grid tile: m = 16*j + i
WH, WW = TH + 4, TW + 4   # window 20 rows x 12 col-strips
NW = WH * WW              # 240
BAND = 4 * WH + 4 + 1     # 85 = span of {20b + a}
NCOLS = BAND + TH - 1     # 100 columns written per (row, tile)
SHIFT0 = TH - 1           # 15
PITCH = SHIFT0 + 12 * NCOLS + 1   # 1216 per-partition scratch row (f16 elems)
PARITIES = ((0, 0), (0, 1), (1, 0), (1, 1))  # (py, px); pairs share py
IN2CH = 24                # in2 chunk rows
IN1CH = 16                # in1 chunk rows


def build_corr(nc, tc, in1_d, in2_d, out_d, scratch_d, C, H, W):
    from concourse import masks

    KC = C // 128
    HW = H * W
    NT = (W // 2) // TW       # 12 tiles per parity row-strip
    SROWS = 2 * TH            # 32 natural rows per strip
    NSTRIP = H // SROWS       # 3
    PKC = (W // 2) + 4        # 100 packed plane cols (+-2 parity halo)
    RPAR = 128 * PITCH        # scratch region per parity
    ULEN = (NT - 1) * NCOLS + BAND  # 1185 readback cols per parity
    NCH2 = H // IN2CH + 1     # 4 in2 chunks (last partial-use)
    NCH1 = H // IN1CH         # 6 in1 chunks
    inv_c = 1.0 / C

    with (
        tc.tile_pool(name="const", bufs=1) as cpool,
        tc.tile_pool(name="in2c", bufs=5) as in2_pool,
        tc.tile_pool(name="in1c", bufs=4) as in1_pool,
        tc.tile_pool(name="pk1", bufs=4) as pk1_pool,
        tc.tile_pool(name="pk2", bufs=3) as pk2_pool,
        tc.tile_pool(name="spool", bufs=2) as s_pool,
        tc.tile_pool(name="upool", bufs=3) as u_pool,
        tc.tile_pool(name="ypool", bufs=3) as y_pool,
        tc.tile_pool(name="opool", bufs=1) as o_pool,
        tc.tile_pool(name="psumw", bufs=6, space="PSUM") as pw_pool,
        tc.tile_pool(name="psum2", bufs=2, space="PSUM") as p2_pool,
    ):
        identity = cpool.tile([128, 128], f16)
        masks.make_identity(nc, identity[:])

        # pk2 sources are f16 now (DVE/ACT 2x faster) and gpsimd also
        # issues the in2 cast loads -> shift pack weight off gpsimd
        PK2_CYCLE = ('g', 'v', 's', 'v', 'g', 'v', 's', 'v')
        pk2_rr = [0]

        def pk2_engine():
            pk2_rr[0] = (pk2_rr[0] + 1) % len(PK2_CYCLE)
            return {'s': nc.scalar, 'v': nc.vector,
                    'g': nc.gpsimd}[PK2_CYCLE[pk2_rr[0]]]

        # per-(chunk, k) tiles so the k=0 packs can start as soon as the
        # k=0 halves land (halves the compute-start latency at prologue)
        chunk_in2 = {}
        chunk_in1 = {}

        def load_in2_chunk(c, k):
            # f32 -> f16 cast in the DMA (SWDGE): halves in2 SBUF footprint
            # and doubles pk2 pack speed on DVE/ACT
            t_ = in2_pool.tile([128, IN2CH, W], f16, tag="in2c",
                               name="in2chunk")
            chunk_in2[(c, k)] = t_
            nc.gpsimd.dma_start(
                t_[:],
                bass.AP(in2_d, k * 128 * HW + c * IN2CH * W,
                        [[HW, 128], [1, IN2CH * W]]))

        def load_in1_chunk(u, k):
            t_ = in1_pool.tile([128, IN1CH, W], f32, tag="in1c",
                               name="in1chunk")
            chunk_in1[(u, k)] = t_
            nc.sync.dma_start(
                t_[:],
                bass.AP(in1_d, k * 128 * HW + u * IN1CH * W,
                        [[HW, 128], [W, IN1CH], [1, W]]))

        def pk2_ranges(s, py):
            # (chunk_idx, start_row_in_chunk, r0, nr) covering r in [0,20)
            # natural row of r: 32s - 4 + py + 2r; chunk c = rows [24c, 24c+24)
            if s == 0:
                return [(0, py, 2, 12), (1, py, 14, 6)]
            if s == 1:
                return [(1, 4 + py, 0, 10), (2, py, 10, 10)]
            return [(2, 12 + py, 0, 6), (3, py, 6, 12)]

        def pack_pk1(s):
            pk1_t = {}
            for pi, (py, px) in enumerate(PARITIES):
                pk1_ = pk1_pool.tile([128, KC, NT, 128], f16, tag="pk1",
                                     name="pk1")
                pk1_t[pi] = pk1_
                dap = pk1_[:]
                p_d = dap.ap[0][0]
                for k in range(KC):
                    for ih in range(2):
                        cap = chunk_in1[(2 * s + ih, k)][:]
                        src = bass.AP(
                            cap.tensor,
                            cap.offset + py * W + px,
                            [[cap.ap[0][0], 128], [2 * TW, NT], [2, TW],
                             [2 * W, TH // 2]])
                        dst = bass.AP(
                            dap.tensor,
                            dap.offset + k * NT * 128 + ih * (TH // 2),
                            [[p_d, 128], [128, NT], [TH, TW], [1, TH // 2]])
                        if (pi + k + ih) % 2 == 0:
                            nc.scalar.copy(dst, src)
                        else:
                            nc.vector.tensor_copy(dst, src)
            return pk1_t

        def pack_pk2(s, pi):
            py, px = PARITIES[pi]
            pk2_ = pk2_pool.tile([128, KC, PKC, WH], f16, tag="pk2",
                                 name="pk2")
            dap = pk2_[:]
            p_d = dap.ap[0][0]
            # zero the +-2 parity col halo (s' in [0,2) and [98,100))
            nc.gpsimd.memset(pk2_[:, :, 0:2, :], 0.0)
            nc.gpsimd.memset(pk2_[:, :, PKC - 2:PKC, :], 0.0)
            # zero rows beyond the image (top of strip 0, bottom of last)
            if s == 0:
                nc.gpsimd.memset(pk2_[:, :, :, 0:2], 0.0)
            if s == NSTRIP - 1:
                nc.gpsimd.memset(pk2_[:, :, :, WH - 2:WH], 0.0)
            for k in range(KC):
                for (ci, row0, r0, nr) in pk2_ranges(s, py):
                    cap = chunk_in2[(ci, k)][:]
                    src = bass.AP(
                        cap.tensor,
                        cap.offset + row0 * W + px,
                        [[cap.ap[0][0], 128], [2, PKC - 4], [2 * W, nr]])
                    dst = bass.AP(
                        dap.tensor,
                        dap.offset + k * PKC * WH + 2 * WH + r0,
                        [[p_d, 128], [WH, PKC - 4], [1, nr]])
                    eng = pk2_engine()
                    if eng is nc.scalar:
                        eng.copy(dst, src)
                    else:
                        eng.tensor_copy(dst, src)
            return pk2_

        def matmuls(pk1_, pk2_, S_, pip):
            # two tiles share one PSUM bank; one 480-el drain copy per pair
            pk2_ap = pk2_[:]
            p_pk2 = pk2_ap.ap[0][0]
            for tp in range(NT // 2):
                pw = pw_pool.tile([128, 2, NW], f32, tag="pw", name="pw")
                for tt in range(2):
                    t = 2 * tp + tt
                    for k in range(KC):
                        rhs = bass.AP(
                            pk2_ap.tensor,
                            pk2_ap.offset + k * PKC * WH + t * TW * WH,
                            [[p_pk2, 128], [1, NW]])
                        nc.tensor.matmul(pw[:, tt, :], pk1_[:, k, t, :], rhs,
                                         start=(k == 0), stop=(k == KC - 1))
                if tp % 2 == 0:
                    nc.vector.tensor_copy(S_[:, pip, 2 * tp:2 * tp + 2, :],
                                          pw[:])
                else:
                    nc.scalar.copy(S_[:, pip, 2 * tp:2 * tp + 2, :], pw[:])

        def shear(s, P, S_):
            pair_base = (s % 2) * (4 * RPAR) + 2 * P * RPAR
            s_ap = S_[:]
            p_S = s_ap.ap[0][0]
            for pip in range(2):
                for j in range(TW):
                    sap = bass.AP(s_ap.tensor,
                                  s_ap.offset + 16 * j * p_S + pip * NT * NW
                                  + WH * j,
                                  [[p_S, 16], [NW, NT], [1, NCOLS]])
                    dap = bass.AP(scratch_d,
                                  pair_base + pip * RPAR + 16 * j * PITCH
                                  + SHIFT0,
                                  [[PITCH - 1, 16], [NCOLS, NT], [1, NCOLS]])
                    nc.sync.dma_start(dap, sap)
            U_ = u_pool.tile([128, 2, ULEN], f16, tag="U", name="U")
            nc.sync.dma_start(
                U_[:],
                bass.AP(scratch_d, pair_base + SHIFT0,
                        [[PITCH, 128], [RPAR, 2], [1, ULEN]]))
            return U_

        o_state = {}

        def post(s, P, U_):
            # compact 1185 -> 25, transpose [m,d]->[d,m], assemble to o_sbuf
            if P == 0:
                o_state[s] = o_pool.tile([25, SROWS, W], f32, tag="o",
                                         name="o_sbuf")
            o_sbuf = o_state[s]
            o_ap = o_sbuf[:]
            p_o = o_ap.ap[0][0]
            u_ap = U_[:]
            p_u = u_ap.ap[0][0]
            for pip in range(2):
                pi = 2 * P + pip
                py, px = PARITIES[pi]
                Y_ = y_pool.tile([128, NT, 25], f16, tag="Y", name="Y")
                src = bass.AP(u_ap.tensor, u_ap.offset + pip * ULEN,
                              [[p_u, 128], [NCOLS, NT], [1, 5], [WH, 5]])
                nc.vector.tensor_copy(Y_[:], src)
                for half in range(2):
                    # 1-bank p2 halves free 2 PSUM banks for pw bufs=6
                    p2 = p2_pool.tile([25, (NT // 2) * 128], f16, tag="p2",
                                      name="p2")
                    for tt in range(NT // 2):
                        t = half * (NT // 2) + tt
                        nc.tensor.transpose(p2[:, tt * 128:(tt + 1) * 128],
                                            Y_[:, t, :], identity[:])
                    # dim order: i outer, tiles, then j innermost so the dst
                    # write stride is 2 elements (4B) instead of 2W (768B)
                    p2_ap = p2[:]
                    asrc = bass.AP(p2_ap.tensor, p2_ap.offset,
                                   [[p2_ap.ap[0][0], 25], [1, TH],
                                    [128, NT // 2], [TH, TW]])
                    adst = bass.AP(o_ap.tensor,
                                   o_ap.offset + py * W + px
                                   + half * (NT // 2) * 2 * TW,
                                   [[p_o, 25], [2 * W, TH],
                                    [2 * TW, NT // 2], [2, TW]])
                    if (pi + half) % 2 == 0:
                        nc.scalar.mul(adst, asrc, inv_c)
                    else:
                        nc.vector.tensor_scalar_mul(adst, asrc, inv_c)

        def out_dma(s):
            nc.sync.dma_start(
                bass.AP(out_d, s * SROWS * W, [[HW, 25], [W, SROWS], [1, W]]),
                o_state[s][:])

        # prologue loads: k=0 halves first so the first packs/matmuls can
        # start before the k=1 halves arrive
        for k in range(KC):
            load_in2_chunk(0, k)
            load_in2_chunk(1, k)
            load_in1_chunk(0, k)
            load_in1_chunk(1, k)

        U_hist = {}
        for s in range(NSTRIP):
            pk1_t = pack_pk1(s)
            if s > 0:
                post(s - 1, 0, U_hist[(s - 1, 0)])
            if s + 1 < NSTRIP:
                for k in range(KC):
                    load_in1_chunk(2 * s + 2, k)
                    load_in1_chunk(2 * s + 3, k)
            for P in range(2):
                S_ = s_pool.tile([128, 2, NT, NW], f16, tag="S", name="S")
                for pip in range(2):
                    pi = 2 * P + pip
                    pk2_ = pack_pk2(s, pi)
                    if pi == 3 and s + 1 < NSTRIP:
                        load_in2_chunk(s + 2, 0)
                        load_in2_chunk(s + 2, 1)
                    matmuls(pk1_t[pi], pk2_, S_, pip)
                U_hist[(s, P)] = shear(s, P, S_)
                if P == 0 and s > 0:
                    post(s - 1, 1, U_hist[(s - 1, 1)])
                    out_dma(s - 1)
                if P == 0 and s == NSTRIP - 1:
                    # tail shrink: last strip's first post overlaps pair 1
                    post(s, 0, U_hist[(s, 0)])
        post(NSTRIP - 1, 1, U_hist[(NSTRIP - 1, 1)])
        out_dma(NSTRIP - 1)


def scratch_elems():
    return 2 * 4 * 128 * PITCH


def build_module(C=256, H=96, W=192):
    import concourse.bacc as bacc
    import concourse.tile as tile
    nc = bacc.Bacc("TRN2", target_bir_lowering=False, debug=False)
    in1_d = nc.dram_tensor("in1", [C, H, W], f32, kind="ExternalInput")
    in2_d = nc.dram_tensor("in2", [C, H, W], f32, kind="ExternalInput")
    out_d = nc.dram_tensor("out", [25, H, W], f32, kind="ExternalOutput")
    scratch_d = nc.dram_tensor("scratch", [scratch_elems()], f16)
    with tile.TileContext(nc) as tc:
        build_corr(nc, tc, in1_d, in2_d, out_d, scratch_d, C, H, W)
    nc.compile()
    return nc


def reference_np(in1, in2, md=4, st=2):
    in1, in2 = in1[None], in2[None]
    B, C, H, W = in1.shape
    in2p = np.pad(in2, ((0, 0), (0, 0), (md, md), (md, md)))
    outs = []
    for dy in range(0, 2 * md + 1, st):
        for dx in range(0, 2 * md + 1, st):
            outs.append((in1 * in2p[:, :, dy:dy + H, dx:dx + W]).mean(axis=1))
    return np.stack(outs, axis=1)[0]


_NC = None


def _get_nc():
    global _NC
    if _NC is None:
        _NC = build_module(C_FULL, H_FULL, W_FULL)
    return _NC


def kernel(in1, in2):
    from concourse.bass_utils import run_bass_kernel_spmd
    in1 = np.ascontiguousarray(np.asarray(in1, dtype=np.float32))
    in2 = np.ascontiguousarray(np.asarray(in2, dtype=np.float32))
    assert in1.shape == (B_FULL, C_FULL, H_FULL, W_FULL), in1.shape
    nc = _get_nc()
    in_maps = [{"in1": in1[b], "in2": in2[b]} for b in range(B_FULL)]
    res = run_bass_kernel_spmd(nc, in_maps, core_ids=list(range(B_FULL)))
    out = np.stack(
        [np.asarray(res.results[b]["out"]).reshape(25, H_FULL, W_FULL)
         for b in range(B_FULL)], axis=0)
    return out

